# revision 3
# baseline (speedup 1.0000x reference)
# Trainium2 Bass kernel for CustomFullyConnectedLayer:
#   y = x @ W.T,  W[(c+i)%N, c] += V[i, c] for i in diag_pos  (banded weight)
# Strategy: data-parallel over batch across 8 cores. On each core:
#   y[b, r] = sum_{c in [r-29, r] mod N} x[b, c] * W[r, c]
# Tiled as 32 output blocks of 96 columns; each block needs a 128-wide
# (125 used) window of x features -> one K=128 matmul per block with a
# host-built band block of W.T. x windows are produced by PE transposes of
# a wrap-extended x tile (x_ext[:, k] = x[:, (k-32) mod N]).
#
# PSUM layout rule: a matmul's output slice must NEVER cross a 2KB PSUM
# bank boundary (bank-crossing writebacks intermittently corrupt on cold
# runs). r-blocks are grouped 8 per PSUM tile shaped [128, 8, 128] f32
# (= exactly 2 banks); slice k sits at byte 512*k, 384B wide, in-bank.
import os
import sys

import numpy as np

if "/opt/trn_rl_repo" not in sys.path:
    sys.path.insert(0, "/opt/trn_rl_repo")

import ml_dtypes

BATCH = 8192
N = 3072
NCORES = 8
BC = BATCH // NCORES          # 1024 rows per core
NBT = BC // 128               # 8 batch tiles per core
RW = 96                       # output r-block width
NRB = N // RW                 # 32 r-blocks
PAD = 32                      # left extension of x (covers band offsets 0..29)
GS = 8                        # r-blocks per psum group (8*512B = 2 banks)
NG = NRB // GS                # 4 groups per btile

_CACHE = {}
LAST_RESULTS = None


def _build_program(dt_flag: str):
    import concourse.mybir as mybir
    import concourse.tile as tile
    from concourse import bacc

    cdt = mybir.dt.float32 if dt_flag == "fp32" else mybir.dt.bfloat16
    f32 = mybir.dt.float32
    ydt = f32 if dt_flag in ("fp32", "f32y") else mybir.dt.bfloat16

    nc = bacc.Bacc("TRN2", target_bir_lowering=False, debug=False)
    xs = nc.dram_tensor("xs", [BC, N + PAD], cdt, kind="ExternalInput")
    wb = nc.dram_tensor("wb", [128, NRB, RW], cdt, kind="ExternalInput")
    ident = nc.dram_tensor("ident", [128, 128], cdt, kind="ExternalInput")
    ys = nc.dram_tensor("ys", [BC, N], ydt, kind="ExternalOutput")

    with tile.TileContext(nc) as tc:
        with (
            tc.tile_pool(name="consts", bufs=1) as consts,
            tc.tile_pool(name="xin", bufs=4) as xin,
            tc.tile_pool(name="xtp", bufs=3) as xtp,
            tc.tile_pool(name="yout", bufs=2) as yout,
            tc.tile_pool(name="ptr", bufs=2, space="PSUM") as ptr,
            tc.tile_pool(name="pyb", bufs=3, space="PSUM") as pyb,
        ):
            id_sb = consts.tile([128, 128], cdt)
            nc.sync.dma_start(out=id_sb, in_=ident[:, :])
            wb_sb = consts.tile([128, NRB, RW], cdt)
            nc.gpsimd.dma_start(out=wb_sb, in_=wb[:, :, :])

            # PE warm-up: dummy matmuls during the DMA fill so the HAM clock
            # gate opens (1.2 -> 2.4 GHz) before the first real transpose.
            wsrc = consts.tile([128, 128], cdt)
            nc.vector.memset(wsrc, 0.0)
            wps = pyb.tile([128, GS, 128], f32, tag="py")
            for _ in range(56):
                nc.tensor.matmul(
                    wps[:, 0, :], lhsT=wsrc, rhs=wsrc, start=True, stop=True
                )

            xsplit = 1600  # transposes rho<16 need cols < 96*15+128 = 1568
            for t in range(NBT):
                rows = slice(t * 128, (t + 1) * 128)
                x_ext = xin.tile([128, N + PAD], cdt)
                if t == 0:
                    # finer split so the first transposes start sooner
                    nc.sync.dma_start(out=x_ext[:, :800], in_=xs[rows, :800])
                    nc.sync.dma_start(
                        out=x_ext[:, 800:xsplit], in_=xs[rows, 800:xsplit]
                    )
                else:
                    nc.sync.dma_start(out=x_ext[:, :xsplit], in_=xs[rows, :xsplit])
                nc.sync.dma_start(out=x_ext[:, xsplit:], in_=xs[rows, xsplit:])

                # transpose 32 feature windows: xT[p, b] = x_ext[b, 96*rho + p]
                tg = 8 if cdt == mybir.dt.bfloat16 else 4  # transposes per bank
                xT = xtp.tile([128, NRB, 128], cdt)
                for g in range(NRB // tg):
                    pt = ptr.tile([128, tg, 128], cdt)
                    for s in range(tg):
                        rho = tg * g + s
                        nc.tensor.transpose(
                            pt[:, s, :], x_ext[:, RW * rho: RW * rho + 128], id_sb
                        )
                    if cdt == mybir.dt.bfloat16:
                        # bitcast to int32: halves element count for the copy
                        nc.vector.tensor_copy(
                            out=xT[:, tg * g:tg * (g + 1), :].bitcast(
                                mybir.dt.int32
                            ),
                            in_=pt.bitcast(mybir.dt.int32),
                        )
                    else:
                        nc.vector.tensor_copy(
                            out=xT[:, tg * g:tg * (g + 1), :], in_=pt
                        )

                y_sb = yout.tile([128, NRB, RW], ydt)
                for g in range(NG):  # groups of 8 r-blocks: 2 psum banks each
                    py = pyb.tile([128, GS, 128], f32, tag="py")
                    for k in range(GS):
                        rho = GS * g + k
                        nc.tensor.matmul(
                            py[:, k, :RW],
                            lhsT=xT[:, rho, :],
                            rhs=wb_sb[:, rho, :],
                            start=True,
                            stop=True,
                        )
                    ydst = y_sb[:, GS * g: GS * (g + 1), :]
                    if g == 0:
                        # balance copy load between ACT and DVE; group 0 is
                        # ready earliest so it can't head-of-line-block the
                        # next btile's xT copies in the DVE queue
                        nc.vector.tensor_copy(out=ydst, in_=py[:, :, :RW])
                    else:
                        nc.scalar.copy(out=ydst, in_=py[:, :, :RW])
                    if t == NBT - 1:
                        # last btile: store per group so the pipeline drain
                        # overlaps the final matmuls instead of serializing
                        nc.gpsimd.dma_start(
                            out=ys[rows, GS * RW * g: GS * RW * (g + 1)],
                            in_=y_sb[:, GS * g: GS * (g + 1), :],
                        )
                if t < NBT - 1:
                    # single store per btile on the idle GPSIMD SWDGE queue,
                    # keeping SP free for x loads
                    nc.gpsimd.dma_start(out=ys[rows, :], in_=y_sb)

    nc.compile()
    return nc


def _host_prep(x, V, diag_pos, dt_flag):
    np_dt = np.float32 if dt_flag == "fp32" else ml_dtypes.bfloat16
    x = np.ascontiguousarray(np.asarray(x, dtype=np.float32))
    V = np.asarray(V, dtype=np.float32)
    diag = np.asarray(diag_pos).astype(np.int64) % N
    if diag.size and int(diag.max()) > PAD:
        raise ValueError(
            f"band kernel supports diag offsets <= {PAD}, got {int(diag.max())}"
        )

    # band[p, rho, q] = W.T[c, r] = W[r, c],  c=(RW*rho-PAD+p)%N, r=RW*rho+q
    # W[(c+i)%N, c] += V[i, c]  ->  band[q+PAD-i, rho, q] += V[i, (r-i)%N]
    band = np.zeros((128, NRB, RW), np.float32)
    rho = np.arange(NRB)[:, None]
    q = np.arange(RW)[None, :]
    for i in diag:
        i = int(i)
        c = (RW * rho + q - i) % N                     # [NRB, RW]
        p = q + PAD - i                                # [1, RW] in [3, 127]
        np.add.at(band, (np.broadcast_to(p, c.shape), rho, q), V[i, c])

    # x_ext[b, k] = x[b, (k - PAD) % N]
    x_ext = np.empty((BATCH, N + PAD), np_dt)
    x_ext[:, PAD:] = x
    x_ext[:, :PAD] = x[:, N - PAD:]

    band = band.astype(np_dt)
    identity = np.eye(128, dtype=np_dt)
    return x_ext, band, identity


def kernel(x, V, diag_pos):
    global LAST_RESULTS
    from concourse.bass_utils import run_bass_kernel_spmd

    dt_flag = os.environ.get("KERNEL_DTYPE", "bf16")
    if dt_flag not in _CACHE:
        _CACHE[dt_flag] = _build_program(dt_flag)
    nc = _CACHE[dt_flag]

    x_ext, band, identity = _host_prep(x, V, diag_pos, dt_flag)
    in_maps = [
        {
            "xs": x_ext[k * BC:(k + 1) * BC],
            "wb": band,
            "ident": identity,
        }
        for k in range(NCORES)
    ]
    res = run_bass_kernel_spmd(nc, in_maps, core_ids=list(range(NCORES)))
    LAST_RESULTS = res
    out = np.concatenate([r["ys"] for r in res.results], axis=0)
    return np.ascontiguousarray(out.astype(np.float32))


# revision 5
# speedup vs baseline: 1.0237x; 1.0237x over previous
# Trainium2 Bass kernel for CustomFullyConnectedLayer:
#   y = x @ W.T,  W[(c+i)%N, c] += V[i, c] for i in diag_pos  (banded weight)
# Strategy: data-parallel over batch across 8 cores. On each core:
#   y[b, r] = sum_{c in [r-29, r] mod N} x[b, c] * W[r, c]
# Tiled as 32 output blocks of 96 columns; each block needs a 128-wide
# (125 used) window of x features -> one K=128 matmul per block with a
# host-built band block of W.T. x windows are produced by PE transposes of
# a wrap-extended x tile (x_ext[:, k] = x[:, (k-32) mod N]).
#
# PSUM layout rule: a matmul's output slice must NEVER cross a 2KB PSUM
# bank boundary (bank-crossing writebacks intermittently corrupt on cold
# runs). r-blocks are grouped 8 per PSUM tile shaped [128, 8, 128] f32
# (= exactly 2 banks); slice k sits at byte 512*k, 384B wide, in-bank.
import os
import sys

import numpy as np

if "/opt/trn_rl_repo" not in sys.path:
    sys.path.insert(0, "/opt/trn_rl_repo")

import ml_dtypes

BATCH = 8192
N = 3072
NCORES = 8
BC = BATCH // NCORES          # 1024 rows per core
NBT = BC // 128               # 8 batch tiles per core
RW = 96                       # output r-block width
NRB = N // RW                 # 32 r-blocks
PAD = 32                      # left extension of x (covers band offsets 0..29)
GS = 8                        # r-blocks per psum group (8*512B = 2 banks)
NG = NRB // GS                # 4 groups per btile

_CACHE = {}
LAST_RESULTS = None


def _build_program(dt_flag: str):
    import concourse.mybir as mybir
    import concourse.tile as tile
    from concourse import bacc

    cdt = mybir.dt.float32 if dt_flag == "fp32" else mybir.dt.bfloat16
    f32 = mybir.dt.float32
    ydt = f32 if dt_flag in ("fp32", "f32y") else mybir.dt.bfloat16

    nc = bacc.Bacc("TRN2", target_bir_lowering=False, debug=False)
    xs = nc.dram_tensor("xs", [BC, N + PAD], cdt, kind="ExternalInput")
    wb = nc.dram_tensor("wb", [128, NRB, RW], cdt, kind="ExternalInput")
    ident = nc.dram_tensor("ident", [128, 128], cdt, kind="ExternalInput")
    ys = nc.dram_tensor("ys", [BC, N], ydt, kind="ExternalOutput")

    with tile.TileContext(nc) as tc:
        with (
            tc.tile_pool(name="consts", bufs=1) as consts,
            tc.tile_pool(name="xin", bufs=4) as xin,
            tc.tile_pool(name="xtp", bufs=3) as xtp,
            tc.tile_pool(name="yout", bufs=2) as yout,
            tc.tile_pool(name="ptr", bufs=2, space="PSUM") as ptr,
            tc.tile_pool(name="pyb", bufs=3, space="PSUM") as pyb,
        ):
            id_sb = consts.tile([128, 128], cdt)
            nc.sync.dma_start(out=id_sb, in_=ident[:, :])

            # btile-0 x chunks issue FIRST so the first transposes aren't
            # gated by other traffic during the cold DMA ramp.
            xsplit = 1600  # transposes rho<16 need cols < 96*15+128 = 1568
            x_ext0 = xin.tile([128, N + PAD], cdt, tag="x_ext")
            nc.sync.dma_start(out=x_ext0[:, :800], in_=xs[0:128, :800])
            nc.sync.dma_start(out=x_ext0[:, 800:xsplit], in_=xs[0:128, 800:xsplit])
            nc.sync.dma_start(out=x_ext0[:, xsplit:], in_=xs[0:128, xsplit:])

            # band weights on the same HWDGE path, after btile-0's x
            wb_sb = consts.tile([128, NRB, RW], cdt)
            nc.sync.dma_start(out=wb_sb, in_=wb[:, :, :])

            # PE warm-up: dummy matmuls during the DMA fill so the HAM clock
            # gate opens (1.2 -> 2.4 GHz) before the first real transpose.
            wsrc = consts.tile([128, 128], cdt)
            nc.vector.memset(wsrc, 0.0)
            wps = pyb.tile([128, GS, 128], f32, tag="py")
            for _ in range(56):
                nc.tensor.matmul(
                    wps[:, 0, :], lhsT=wsrc, rhs=wsrc, start=True, stop=True
                )

            for t in range(NBT):
                rows = slice(t * 128, (t + 1) * 128)
                if t == 0:
                    x_ext = x_ext0
                else:
                    # single full-width DMA: one 6208B descriptor per
                    # partition halves descriptor overhead vs two chunks
                    x_ext = xin.tile([128, N + PAD], cdt, tag="x_ext")
                    nc.sync.dma_start(out=x_ext, in_=xs[rows, :])

                # transpose 32 feature windows: xT[p, b] = x_ext[b, 96*rho + p]
                tg = 8 if cdt == mybir.dt.bfloat16 else 4  # transposes per bank
                xT = xtp.tile([128, NRB, 128], cdt)
                for g in range(NRB // tg):
                    pt = ptr.tile([128, tg, 128], cdt)
                    for s in range(tg):
                        rho = tg * g + s
                        nc.tensor.transpose(
                            pt[:, s, :], x_ext[:, RW * rho: RW * rho + 128], id_sb
                        )
                    if cdt == mybir.dt.bfloat16:
                        # bitcast to int32: halves element count for the copy
                        nc.vector.tensor_copy(
                            out=xT[:, tg * g:tg * (g + 1), :].bitcast(
                                mybir.dt.int32
                            ),
                            in_=pt.bitcast(mybir.dt.int32),
                        )
                    else:
                        nc.vector.tensor_copy(
                            out=xT[:, tg * g:tg * (g + 1), :], in_=pt
                        )

                y_sb = yout.tile([128, NRB, RW], ydt)
                for g in range(NG):  # groups of 8 r-blocks: 2 psum banks each
                    py = pyb.tile([128, GS, 128], f32, tag="py")
                    for k in range(GS):
                        rho = GS * g + k
                        nc.tensor.matmul(
                            py[:, k, :RW],
                            lhsT=xT[:, rho, :],
                            rhs=wb_sb[:, rho, :],
                            start=True,
                            stop=True,
                        )
                    ydst = y_sb[:, GS * g: GS * (g + 1), :]
                    if g == 0:
                        # balance copy load between ACT and DVE; group 0 is
                        # ready earliest so it can't head-of-line-block the
                        # next btile's xT copies in the DVE queue
                        nc.vector.tensor_copy(out=ydst, in_=py[:, :, :RW])
                    else:
                        nc.scalar.copy(out=ydst, in_=py[:, :, :RW])
                    if t >= NBT - 2:
                        # last btiles: store per group on the HWDGE path
                        # (x loads done by then) so the drain overlaps the
                        # final matmuls instead of serializing
                        nc.sync.dma_start(
                            out=ys[rows, GS * RW * g: GS * RW * (g + 1)],
                            in_=y_sb[:, GS * g: GS * (g + 1), :],
                        )
                if t < NBT - 2:
                    if t < 4:
                        # early btiles: GPSIMD SWDGE queue, keeping SP free
                        # for x loads which pace the pipeline
                        nc.gpsimd.dma_start(out=ys[rows, :], in_=y_sb)
                    else:
                        # late btiles: x issue backlog is done; use HWDGE
                        nc.sync.dma_start(out=ys[rows, :], in_=y_sb)

    nc.compile()
    return nc


def _host_prep(x, V, diag_pos, dt_flag):
    np_dt = np.float32 if dt_flag == "fp32" else ml_dtypes.bfloat16
    x = np.ascontiguousarray(np.asarray(x, dtype=np.float32))
    V = np.asarray(V, dtype=np.float32)
    diag = np.asarray(diag_pos).astype(np.int64) % N
    if diag.size and int(diag.max()) > PAD:
        raise ValueError(
            f"band kernel supports diag offsets <= {PAD}, got {int(diag.max())}"
        )

    # band[p, rho, q] = W.T[c, r] = W[r, c],  c=(RW*rho-PAD+p)%N, r=RW*rho+q
    # W[(c+i)%N, c] += V[i, c]  ->  band[q+PAD-i, rho, q] += V[i, (r-i)%N]
    band = np.zeros((128, NRB, RW), np.float32)
    rho = np.arange(NRB)[:, None]
    q = np.arange(RW)[None, :]
    for i in diag:
        i = int(i)
        c = (RW * rho + q - i) % N                     # [NRB, RW]
        p = q + PAD - i                                # [1, RW] in [3, 127]
        np.add.at(band, (np.broadcast_to(p, c.shape), rho, q), V[i, c])

    # x_ext[b, k] = x[b, (k - PAD) % N]
    x_ext = np.empty((BATCH, N + PAD), np_dt)
    x_ext[:, PAD:] = x
    x_ext[:, :PAD] = x[:, N - PAD:]

    band = band.astype(np_dt)
    identity = np.eye(128, dtype=np_dt)
    return x_ext, band, identity


def kernel(x, V, diag_pos):
    global LAST_RESULTS
    from concourse.bass_utils import run_bass_kernel_spmd

    dt_flag = os.environ.get("KERNEL_DTYPE", "bf16")
    if dt_flag not in _CACHE:
        _CACHE[dt_flag] = _build_program(dt_flag)
    nc = _CACHE[dt_flag]

    x_ext, band, identity = _host_prep(x, V, diag_pos, dt_flag)
    in_maps = [
        {
            "xs": x_ext[k * BC:(k + 1) * BC],
            "wb": band,
            "ident": identity,
        }
        for k in range(NCORES)
    ]
    res = run_bass_kernel_spmd(nc, in_maps, core_ids=list(range(NCORES)))
    LAST_RESULTS = res
    out = np.concatenate([r["ys"] for r in res.results], axis=0)
    return np.ascontiguousarray(out.astype(np.float32))
